# revision 1
# baseline (speedup 1.0000x reference)
"""HeterogeneousKANLayer forward on 8 Trainium2 NeuronCores.

Math (reference):
  xn    = tanh(x)                                  [B, I]
  base  = silu(xn)                                 [B, I]
  basis = exp(-((xn - c_j)/w)^2), c_j evenly spaced on [-1,1], w = 2/(C-1)
  out[b,o] = sum_{i,c} basis[b,i,c]*coef[i,o,c]*scale_sp[o,i]
           + sum_i base[b,i]*scale_base[o,i]

Kernel strategy (data-parallel over batch, 8 cores x 512 rows):
  Everything is one [512b, 5632k] @ [5632k, 512o] matmul per core, where
  k = (center, i) channels plus one silu channel group. Host folds
  scale_sp into coef and appends scale_base^T as the last 4 k-tiles.
  Gaussian per center j is computed as u * exp(a_j*xn + b_j) with
  u = exp(-xn^2/w^2): one ACT exp + one DVE multiply per center
  (ACT ~25us, DVE ~25us, PE ~37.5us bf16 - PE-bound at the fp32r/bf16
  roofline for this shard).
"""

import sys
import types

import numpy as np
import ml_dtypes

import concourse.bass as bass
import concourse.tile as tile
from concourse import bacc, mybir

N_CORES = 8
B = 4096
I = 512
O = 512
C = 10
BS = B // N_CORES          # batch rows per core (512)
W_SPACING = 2.0 / (C - 1)  # rbf width == center spacing
INV_W2 = 1.0 / (W_SPACING * W_SPACING)  # 20.25
NT = I // 128              # 4 i-tiles
NKT = NT * (C + 1)         # 44 k-tiles of 128 (10 centers + silu)

_CACHE = {}


def _build():
    """Build and finalize the per-core Bass module (same on all cores)."""
    nc = bacc.Bacc("TRN2", target_bir_lowering=False, debug=False,
                   num_devices=N_CORES)
    f32 = mybir.dt.float32
    bf16 = mybir.dt.bfloat16
    xt_d = nc.dram_tensor("xt", (I, BS), f32, kind="ExternalInput")
    w_d = nc.dram_tensor("w", (NKT, 128, O), bf16, kind="ExternalInput")
    out_d = nc.dram_tensor("out", (BS, O), f32, kind="ExternalOutput")

    centers = np.linspace(-1.0, 1.0, C)

    with tile.TileContext(nc) as tc:
        with (
            tc.tile_pool(name="big", bufs=1) as big,
            tc.tile_pool(name="wpool", bufs=1) as wpool,
            tc.tile_pool(name="psum", bufs=1, space="PSUM") as psum,
        ):
            # ---- warm-ups: pull ACT table loads + PE HAM ramp off the
            # critical path (tiny ops on a scratch tile at t~0) ----
            warm = big.tile([128, 8], f32, tag="warm")
            nc.vector.memset(warm[:], 0.0)
            for fn in (mybir.ActivationFunctionType.Silu,
                       mybir.ActivationFunctionType.Tanh,
                       mybir.ActivationFunctionType.Exp):
                nc.scalar.activation(out=warm[:], in_=warm[:], func=fn)
            warm_w = wpool.tile([128, O], bf16, tag="warmw")
            nc.vector.memset(warm_w[:], 0.0)
            ps_warm = psum.tile([128, O], mybir.dt.float32, name="pswarm")
            for _ in range(96):
                nc.tensor.matmul(ps_warm[:, :128], warm_w[:, :128],
                                 warm_w[:, :128], start=True, stop=True)

            # ---- load x^T FIRST (it heads the critical path) ----
            xt_sb = big.tile([128, NT * BS], f32, tag="xt")
            for t in range(NT):
                nc.sync.dma_start(out=xt_sb[:, t * BS:(t + 1) * BS],
                                  in_=xt_d[t * 128:(t + 1) * 128, :])

            # ---- weights stream behind xt, in 4-ktile chunks ----
            w_sb = wpool.tile([128, NKT, O], bf16)
            for j in range(NKT // 4):
                nc.sync.dma_start(
                    out=w_sb[:, 4 * j:4 * (j + 1), :],
                    in_=w_d[4 * j:4 * (j + 1)].rearrange("k p o -> p k o"))

            bias_sb = big.tile([128, C], f32, tag="bias")
            for j in range(C):
                b_j = -centers[j] * centers[j] * INV_W2
                nc.vector.memset(bias_sb[:, j:j + 1], float(b_j))

            # ---- g_0 path chunked at [128,512] so the PE can start on
            # (c=0, t=0) as soon as possible ----
            def sl(t):
                return slice(t * BS, (t + 1) * BS)

            a0 = 2.0 * centers[0] * INV_W2
            a1 = 2.0 * centers[1] * INV_W2
            xn = big.tile([128, NT * BS], f32, tag="xn")
            sq = big.tile([128, NT * BS], f32, tag="sq")
            e_0 = big.tile([128, NT * BS], f32, tag="e0")
            e_1c = big.tile([128, NT * BS], f32, tag="e1c")
            u = big.tile([128, NT * BS], f32, tag="u")
            g_0 = big.tile([128, NT * BS], bf16, tag="g0")
            g_1 = big.tile([128, NT * BS], bf16, tag="g1c")
            for t in range(NT):
                nc.scalar.activation(out=xn[:, sl(t)], in_=xt_sb[:, sl(t)],
                                     func=mybir.ActivationFunctionType.Tanh)
                nc.vector.tensor_mul(out=sq[:, sl(t)], in0=xn[:, sl(t)],
                                     in1=xn[:, sl(t)])
                nc.scalar.activation(out=e_0[:, sl(t)], in_=xn[:, sl(t)],
                                     func=mybir.ActivationFunctionType.Exp,
                                     bias=bias_sb[:, 0:1], scale=float(a0))
                nc.scalar.activation(out=u[:, sl(t)], in_=sq[:, sl(t)],
                                     func=mybir.ActivationFunctionType.Exp,
                                     scale=-INV_W2)
                nc.vector.tensor_mul(out=g_0[:, sl(t)], in0=u[:, sl(t)],
                                     in1=e_0[:, sl(t)])
                nc.scalar.activation(out=e_1c[:, sl(t)], in_=xn[:, sl(t)],
                                     func=mybir.ActivationFunctionType.Exp,
                                     bias=bias_sb[:, 1:2], scale=float(a1))
                nc.vector.tensor_mul(out=g_1[:, sl(t)], in0=u[:, sl(t)],
                                     in1=e_1c[:, sl(t)])

            # ---- remaining centers full-tile; silu last (consumed last) ----
            basis = [g_0, g_1]
            for j in range(2, C):
                a_j = 2.0 * centers[j] * INV_W2
                e_j = big.tile([128, NT * BS], f32, tag=f"e{1 + (j % 2)}")
                nc.scalar.activation(out=e_j[:], in_=xn[:],
                                     func=mybir.ActivationFunctionType.Exp,
                                     bias=bias_sb[:, j:j + 1], scale=float(a_j))
                g_j = big.tile([128, NT * BS], bf16, tag=f"g{j}")
                nc.vector.tensor_mul(out=g_j[:], in0=u[:], in1=e_j[:])
                basis.append(g_j)
            silu_bf = big.tile([128, NT * BS], bf16, tag="silu")
            nc.scalar.activation(out=silu_bf[:], in_=xn[:],
                                 func=mybir.ActivationFunctionType.Silu)
            basis.append(silu_bf)  # channel 10 = residual/silu

            # ---- the big matmul: out[b,o] += basis_k[b] * w[k,o] ----
            ps = [psum.tile([128, O], mybir.dt.float32, name=f"ps{bt}")
                  for bt in range(4)]
            for j in range(C):
                for t in range(NT):
                    kt = 4 * j + t
                    for bt in range(4):
                        nc.tensor.matmul(
                            ps[bt],
                            basis[j][:, t * BS + bt * 128: t * BS + (bt + 1) * 128],
                            w_sb[:, kt, :],
                            start=(kt == 0),
                            stop=False,
                        )
            # last k-group bank-major so bank bt finishes (and drains) early
            for bt in range(4):
                for t in range(NT):
                    kt = 4 * C + t
                    nc.tensor.matmul(
                        ps[bt],
                        basis[C][:, t * BS + bt * 128: t * BS + (bt + 1) * 128],
                        w_sb[:, kt, :],
                        start=False,
                        stop=(t == NT - 1),
                    )
                o_sb = big.tile([128, O], f32, tag=f"o{bt}")
                if bt % 2 == 0:
                    nc.scalar.copy(out=o_sb[:], in_=ps[bt][:])
                else:
                    nc.vector.tensor_copy(out=o_sb[:], in_=ps[bt][:])
                nc.sync.dma_start(out=out_d[bt * 128:(bt + 1) * 128, :],
                                  in_=o_sb[:])
    nc.finalize()
    return nc


def _prep_inputs(x, coef, scale_base, scale_sp):
    """Host-side shard + layout prep (cheap numpy reshapes/casts)."""
    x = np.asarray(x, dtype=np.float32)
    coef = np.asarray(coef, dtype=np.float32)
    scale_base = np.asarray(scale_base, dtype=np.float32)
    scale_sp = np.asarray(scale_sp, dtype=np.float32)

    # W[k, o]: k ordered as (center j, i_tile t) then 4 silu tiles.
    wf = coef * scale_sp.T[:, :, None]          # [I, O, C]
    wf = wf.reshape(NT, 128, O, C).transpose(3, 0, 1, 2)  # [C, NT, 128, O]
    w_all = np.concatenate(
        [wf.reshape(C * NT, 128, O), scale_base.T.reshape(NT, 128, O)], axis=0
    ).astype(ml_dtypes.bfloat16)                 # [NKT, 128, O]

    in_maps = []
    for k in range(N_CORES):
        xs = x[k * BS:(k + 1) * BS, :]           # [BS, I]
        in_maps.append({"xt": np.ascontiguousarray(xs.T), "w": w_all})
    return in_maps


def _run(in_maps, trace=False):
    if "antenv.axon_hooks" not in sys.modules:
        try:
            from trn_agent_boot.trn_boot import _ntff_profile_via_ctypes
            _hook = _ntff_profile_via_ctypes("/opt/axon/libaxon_pjrt.so")
            _mod = types.ModuleType("antenv.axon_hooks")
            _mod.get_axon_ntff_profile_hook = lambda: _hook
            sys.modules["antenv.axon_hooks"] = _mod
        except Exception:
            pass
    from concourse.bass_utils import run_bass_kernel_spmd

    if "nc" not in _CACHE:
        _CACHE["nc"] = _build()
    return run_bass_kernel_spmd(_CACHE["nc"], in_maps,
                                core_ids=list(range(N_CORES)), trace=trace)


def kernel(x, coef, scale_base, scale_sp):
    in_maps = _prep_inputs(x, coef, scale_base, scale_sp)
    res = _run(in_maps, trace=False)
    out = np.concatenate([res.results[k]["out"] for k in range(N_CORES)],
                         axis=0)
    return out.astype(np.float32)



# revision 14
# speedup vs baseline: 1.1558x; 1.1558x over previous
"""HeterogeneousKANLayer forward on 8 Trainium2 NeuronCores.

Math (reference):
  xn    = tanh(x)                                  [B, I]
  base  = silu(xn)                                 [B, I]
  basis = exp(-((xn - c_j)/w)^2), c_j evenly spaced on [-1,1], w = 2/(C-1)
  out[b,o] = sum_{i,c} basis[b,i,c]*coef[i,o,c]*scale_sp[o,i]
           + sum_i base[b,i]*scale_base[o,i]

Kernel strategy (data-parallel over batch, 8 cores x 512 rows):
  One [512b, 5632k] @ [5632k, 512o] contraction per core, where
  k = (center, i) channels plus the silu residual.
  The 10 Gaussian-center channel groups run as fp8e4 DoubleRow matmuls
  (K=256 per instruction, ~2x PE rate). Planes are stored scaled by S
  to keep the fp8 band normal; 1/S is folded into the PSUM drain.
  Basis production:
    - anchors j in {0,3,6,8}: q_j = a_j*xn - 20.25*xn^2 (DVE stt, fp16),
      g_j = Exp(q_j + b_j + lnS) on ACT directly to fp8.
    - chains (<=2 hops): g_{j+1} = (r * e^{8-2j}) * g_j on DVE (one on
      GPSIMD), with r = exp(9*xn). Evenly spaced centers make
      consecutive Gaussians differ by exp(9*xn + const).
  silu residual: silu(xn) = 0.5*xn*(1+tanh(xn/2)) -- tanh lives in the
  same ACT table set as exp, so the whole kernel needs ONE table load.
  When scale_base is rank-1 (all rows equal, as with default ones), the
  residual collapses to a per-row scalar v[b] = sum_i silu*sbv[i],
  computed with 4 thin matmuls and added as a per-partition bias during
  the PSUM drain. Otherwise fall back to dense fp16 residual matmuls.
"""

import sys
import types

import numpy as np
import ml_dtypes

import concourse.bass as bass
import concourse.tile as tile
from concourse import bacc, mybir

N_CORES = 8
B = 4096
I = 512
O = 512
C = 10
BS = B // N_CORES          # batch rows per core (512)
NT = I // 128              # 4 i-tiles
W_SP = 2.0 / (C - 1)       # rbf width == center spacing (2/9)
IW2 = 1.0 / (W_SP * W_SP)  # 20.25
CENTERS = np.linspace(-1.0, 1.0, C)
A_J = 2.0 * CENTERS * IW2
B_J = -(CENTERS ** 2) * IW2
DB = B_J[1:] - B_J[:-1]              # 8,6,4,2,0,-2,-4,-6,-8
S_G = 64.0                           # fp8 plane scale
LN_S = float(np.log(S_G))

ANCHORS = [0, 3, 6, 8]               # produced via ACT Exp
DVE_CHAIN = [(1, 0), (2, 1), (4, 3), (7, 6), (9, 8)]
GPS_CHAIN = [(5, 4)]

_CACHE = {}
_DEBUG_TAPS = False


def _build(rank1):
    """Build and finalize the per-core Bass module (same on all cores)."""
    nc = bacc.Bacc("TRN2", target_bir_lowering=False, debug=False,
                   num_devices=N_CORES)
    f32 = mybir.dt.float32
    f16 = mybir.dt.float16
    fp8 = mybir.dt.float8e4
    DR = mybir.MatmulPerfMode.DoubleRow
    MUL = mybir.AluOpType.mult
    ADD = mybir.AluOpType.add
    EXP = mybir.ActivationFunctionType.Exp
    TANH = mybir.ActivationFunctionType.Tanh
    IDENT = mybir.ActivationFunctionType.Identity

    xt_d = nc.dram_tensor("xt", (I, BS), f32, kind="ExternalInput")
    wf_d = nc.dram_tensor("wf", (4 * C, 128, O), fp8, kind="ExternalInput")
    if rank1:
        sbv_d = nc.dram_tensor("sbv", (I,), f16, kind="ExternalInput")
    else:
        ws_d = nc.dram_tensor("ws", (NT, 128, O), f16, kind="ExternalInput")
    out_d = nc.dram_tensor("out", (BS, O), f32, kind="ExternalOutput")

    with tile.TileContext(nc) as tc:
        with (
            tc.tile_pool(name="big", bufs=1) as big,
            tc.tile_pool(name="wpool", bufs=1) as wpool,
            tc.tile_pool(name="psum", bufs=1, space="PSUM") as psum,
        ):
            xt_sb = big.tile([128, NT, BS], f32, tag="xt")
            xn = big.tile([128, NT, BS], f16, tag="xn")
            sqp = big.tile([128, NT, BS], f16, tag="sqp")
            qs = {j: big.tile([128, NT, BS], f16, name=f"q{j}")
                  for j in ANCHORS}
            r_t = big.tile([128, NT, BS], f16, tag="r")
            th2 = big.tile([128, NT, BS], f16, tag="th2")
            silu = big.tile([128, NT, BS], f16, tag="silu")
            g = [big.tile([128, NT, BS], fp8, name=f"g{j}") for j in range(C)]
            wf_sb = wpool.tile([128, 4 * C, O], fp8, name="wf_sb")
            if rank1:
                sbv_sb = wpool.tile([128, NT], f16, name="sbv_sb")
                v16 = big.tile([1, BS], f16, tag="v16")
                ones_sb = big.tile([1, O], f16, tag="ones_sb")
                nc.vector.memset(ones_sb[:], 1.0)
            else:
                ws_sb = wpool.tile([128, NT, O], f16, name="ws_sb")
            warm = big.tile([128, 8], f32, tag="warm")
            bias_sb = big.tile([128, len(ANCHORS)], f32, tag="bias")
            for ai, j in enumerate(ANCHORS):
                nc.vector.memset(bias_sb[:, ai:ai + 1], float(B_J[j] + LN_S))

            def hs(h):
                return slice(2 * h, 2 * h + 2)

            # tiny ACT warm-up: trigger the exp-set table load at t=0
            nc.vector.memset(warm[:], 0.0)
            nc.scalar.activation(out=warm[:], in_=warm[:], func=EXP)
            nc.scalar.activation(out=warm[:], in_=warm[:], func=TANH)
            if not rank1:
                nc.scalar.activation(out=warm[:], in_=warm[:],
                                     func=mybir.ActivationFunctionType.Silu)

            # ---- DMAs ----
            nc.sync.dma_start(
                out=xt_sb[:, 0:2, :],
                in_=xt_d[0:256, :].rearrange("(t p) b -> p t b", p=128))
            nc.sync.dma_start(
                out=xt_sb[:, 2:4, :],
                in_=xt_d[256:512, :].rearrange("(t p) b -> p t b", p=128))
            if rank1:
                nc.sync.dma_start(
                    out=sbv_sb[:, :],
                    in_=sbv_d[:].rearrange("(t p) -> p t", p=128))
            else:
                nc.sync.dma_start(
                    out=ws_sb[:, :, :],
                    in_=ws_d[:].rearrange("k p o -> p k o"))
            for ch in range(5):
                nc.sync.dma_start(
                    out=wf_sb[:, 8 * ch:8 * ch + 8, :],
                    in_=wf_d[8 * ch:8 * ch + 8].rearrange("k p o -> p k o"))

            # ---- ACT: tanh, anchor exps, r, th2 (one table set) ----
            for h in (0, 1):
                nc.scalar.activation(out=xn[:, hs(h), :],
                                     in_=xt_sb[:, hs(h), :], func=TANH)
            # DVE prerequisites for the first anchor
            for h in (0, 1):
                nc.vector.scalar_tensor_tensor(
                    out=sqp[:, hs(h), :], in0=xn[:, hs(h), :],
                    scalar=float(-IW2), in1=xn[:, hs(h), :],
                    op0=MUL, op1=MUL)
            for j in ANCHORS:
                for h in (0, 1):
                    nc.vector.scalar_tensor_tensor(
                        out=qs[j][:, hs(h), :], in0=xn[:, hs(h), :],
                        scalar=float(A_J[j]), in1=sqp[:, hs(h), :],
                        op0=MUL, op1=ADD)

            def anchor(j):
                ai = ANCHORS.index(j)
                for h in (0, 1):
                    nc.scalar.activation(out=g[j][:, hs(h), :],
                                         in_=qs[j][:, hs(h), :], func=EXP,
                                         bias=bias_sb[:, ai:ai + 1])

            anchor(0)
            for h in (0, 1):
                nc.scalar.activation(out=r_t[:, hs(h), :],
                                     in_=xn[:, hs(h), :], func=EXP,
                                     scale=float(2.0 / W_SP))
            for h in (0, 1):
                nc.scalar.activation(out=th2[:, hs(h), :],
                                     in_=xn[:, hs(h), :], func=TANH,
                                     scale=0.5)
            anchor(3)
            anchor(6)
            anchor(8)
            if not rank1:
                for h in (0, 1):
                    nc.scalar.activation(
                        out=silu[:, hs(h), :], in_=xn[:, hs(h), :],
                        func=mybir.ActivationFunctionType.Silu)

            # ---- DVE chains + silu' = xn*(1+th2) = 2*silu ----
            def chain_op(eng, j, src, h):
                eng.scalar_tensor_tensor(
                    out=g[j][:, hs(h), :], in0=r_t[:, hs(h), :],
                    scalar=float(np.exp(DB[src])), in1=g[src][:, hs(h), :],
                    op0=MUL, op1=MUL)

            for h in (0, 1):
                chain_op(nc.vector, 1, 0, h)
            for h in (0, 1):
                chain_op(nc.vector, 2, 1, h)
            if rank1:
                for h in (0, 1):
                    nc.vector.scalar_tensor_tensor(
                        out=silu[:, hs(h), :], in0=th2[:, hs(h), :],
                        scalar=1.0, in1=xn[:, hs(h), :], op0=ADD, op1=MUL)
            for h in (0, 1):
                chain_op(nc.vector, 4, 3, h)
            for h in (0, 1):
                # GPSIMD lacks scalar_tensor_tensor; DB[4]=0 so the
                # scale is exactly 1 and a plain multiply suffices.
                assert float(np.exp(DB[4])) == 1.0
                nc.gpsimd.tensor_tensor(
                    out=g[5][:, hs(h), :], in0=r_t[:, hs(h), :],
                    in1=g[4][:, hs(h), :], op=MUL)
            for h in (0, 1):
                chain_op(nc.vector, 7, 6, h)
            for h in (0, 1):
                chain_op(nc.vector, 9, 8, h)

            # ---- PE ----
            ps = [psum.tile([128, O], f32, name=f"ps{bt}") for bt in range(4)]
            if rank1:
                v_ps = psum.tile([1, BS], f32, name="v_ps")

            def mm_g(j, p, start=False, stop=False):
                for bt in range(4):
                    nc.tensor.matmul(
                        ps[bt],
                        g[j][:, 2 * p:2 * p + 2, bt * 128:(bt + 1) * 128],
                        wf_sb[:, 4 * j + 2 * p:4 * j + 2 * p + 2, :],
                        start=start, stop=stop, perf_mode=DR)

            def mm_silu(t):
                for bt in range(4):
                    nc.tensor.matmul(
                        ps[bt], silu[:, t, bt * 128:(bt + 1) * 128],
                        ws_sb[:, t, :], start=False, stop=False)

            mm_g(0, 0, start=True)
            mm_g(0, 1)
            for (j, p) in [(1, 0), (1, 1), (2, 0), (2, 1), (3, 0), (3, 1)]:
                mm_g(j, p)
            if rank1:
                # v[b] = sum_i silu'*sbv: thin matmuls into a [1, BS] bank
                for t in range(NT):
                    nc.tensor.matmul(v_ps, sbv_sb[:, t:t + 1],
                                     silu[:, t, :],
                                     start=(t == 0), stop=(t == NT - 1))
            else:
                for t in range(NT):
                    mm_silu(t)
            for (j, p) in [(4, 0), (4, 1), (6, 0), (6, 1), (7, 0), (5, 0),
                           (7, 1), (8, 0), (5, 1), (8, 1)]:
                mm_g(j, p)
            if rank1:
                # fold v back in via PE: out[b,o] += v[b]*1 (K=1 matmul,
                # the [1, BS] row is directly the stationary operand)
                nc.vector.tensor_copy(out=v16[:, :], in_=v_ps[:, :])
                for bt in range(4):
                    nc.tensor.matmul(
                        ps[bt], v16[0:1, bt * 128:(bt + 1) * 128],
                        ones_sb[0:1, :], start=False, stop=False)
            mm_g(9, 0)
            # last group bank-major so bank bt drains early
            inv_s = float(1.0 / S_G)
            for bt in range(4):
                nc.tensor.matmul(
                    ps[bt], g[9][:, 2:4, bt * 128:(bt + 1) * 128],
                    wf_sb[:, 38:40, :], start=False, stop=True, perf_mode=DR)
                o_sb = big.tile([128, O], f32, name=f"o{bt}")
                if bt % 2 == 0:
                    nc.scalar.mul(out=o_sb[:], in_=ps[bt][:], mul=inv_s)
                else:
                    nc.vector.tensor_scalar_mul(out=o_sb[:], in0=ps[bt][:],
                                                scalar1=inv_s)
                nc.sync.dma_start(out=out_d[bt * 128:(bt + 1) * 128, :],
                                  in_=o_sb[:])
            if _DEBUG_TAPS:
                taps = {"g0": g[0], "g1": g[1], "g5": g[5], "g9": g[9],
                        "silu": silu, "r": r_t}
                for nm, t_sb in taps.items():
                    dt_ = t_sb.dtype
                    d_out = nc.dram_tensor(f"dbg_{nm}", (128, NT, BS), dt_,
                                           kind="ExternalOutput")
                    nc.sync.dma_start(out=d_out[:, :, :], in_=t_sb[:, :, :])
    nc.finalize()
    return nc


def _prep_inputs(x, coef, scale_base, scale_sp):
    """Host-side shard + layout prep (cheap numpy reshapes/casts)."""
    x = np.asarray(x, dtype=np.float32)
    coef = np.asarray(coef, dtype=np.float32)
    scale_base = np.asarray(scale_base, dtype=np.float32)
    scale_sp = np.asarray(scale_sp, dtype=np.float32)
    rank1 = bool(np.all(scale_base == scale_base[0:1, :]))

    # wf[kt, p, o] with kt = 4*j + t, j = center, t = i-tile.
    wfull = coef * scale_sp.T[:, :, None]                    # [I, O, C]
    wfull = wfull.reshape(NT, 128, O, C).transpose(3, 0, 1, 2)  # [C,NT,128,O]
    wf = np.clip(wfull.reshape(4 * C, 128, O), -240.0, 240.0).astype(
        ml_dtypes.float8_e4m3)

    in_maps = []
    for k in range(N_CORES):
        xs = x[k * BS:(k + 1) * BS, :]                       # [BS, I]
        m = {"xt": np.ascontiguousarray(xs.T), "wf": wf}
        if rank1:
            # silu' = 2*silu; the PSUM carries S_G everywhere, so fold
            # 0.5*S_G into sbv (v then scales down with the drain's 1/S).
            m["sbv"] = (0.5 * S_G * scale_base[0, :]).astype(np.float16)
        else:
            m["ws"] = (S_G * scale_base.T.reshape(NT, 128, O)).astype(
                np.float16)
        in_maps.append(m)
    return in_maps, rank1


def _run(in_maps, rank1, trace=False):
    if "antenv.axon_hooks" not in sys.modules:
        try:
            from trn_agent_boot.trn_boot import _ntff_profile_via_ctypes
            _hook = _ntff_profile_via_ctypes("/opt/axon/libaxon_pjrt.so")
            _mod = types.ModuleType("antenv.axon_hooks")
            _mod.get_axon_ntff_profile_hook = lambda: _hook
            sys.modules["antenv.axon_hooks"] = _mod
        except Exception:
            pass
    from concourse.bass_utils import run_bass_kernel_spmd

    key = ("nc", rank1)
    if key not in _CACHE:
        _CACHE[key] = _build(rank1)
    return run_bass_kernel_spmd(_CACHE[key], in_maps,
                                core_ids=list(range(N_CORES)), trace=trace)


def kernel(x, coef, scale_base, scale_sp):
    in_maps, rank1 = _prep_inputs(x, coef, scale_base, scale_sp)
    res = _run(in_maps, rank1, trace=False)
    out = np.concatenate([res.results[k]["out"] for k in range(N_CORES)],
                         axis=0)
    return out.astype(np.float32)


# revision 16
# speedup vs baseline: 1.1559x; 1.0001x over previous
"""HeterogeneousKANLayer forward on 8 Trainium2 NeuronCores.

Math (reference):
  xn    = tanh(x)                                  [B, I]
  base  = silu(xn)                                 [B, I]
  basis = exp(-((xn - c_j)/w)^2), c_j evenly spaced on [-1,1], w = 2/(C-1)
  out[b,o] = sum_{i,c} basis[b,i,c]*coef[i,o,c]*scale_sp[o,i]
           + sum_i base[b,i]*scale_base[o,i]

Kernel strategy (data-parallel over batch, 8 cores x 512 rows):
  One [512b, 5632k] @ [5632k, 512o] contraction per core.
  The 10 Gaussian-center channel groups run as fp8e4 DoubleRow matmuls
  (K=256 per instruction, ~2x PE rate). Planes are stored scaled by S_G
  to keep the fp8 band normal; 1/S_G is folded into the PSUM drain.
  Basis production (balanced across ACT/DVE/GPSIMD):
    - anchor 0 fully on ACT: Square(xn+1) then Exp(-20.25*sq + lnS).
    - anchors {3,6,8}: q_j = a_j*xn - 20.25*xn^2 on DVE (stt, fp16),
      g_j = Exp(q_j + b_j + lnS) on ACT directly to fp8.
    - chains (<=2 hops): g_{j+1} = (r * e^{8-2j}) * g_j on DVE; the
      (5<-4) step has unit scale and runs as a plain multiply on
      GPSIMD (warmed by a tiny op at t=0). r = exp(9*xn).
  silu residual: silu(xn) = 0.5*xn*(1+tanh(xn/2)) -- tanh shares the
  exp ACT table set, so the whole kernel needs ONE table load.
  When scale_base is rank-1 (all rows equal, e.g. default ones), the
  residual collapses to v[b] = sum_i silu*sbv[i], computed with 4 thin
  matmuls + one K=1 matmul against a ones-row. Otherwise fall back to
  dense fp16 residual matmuls.
"""

import sys
import types

import numpy as np
import ml_dtypes

import concourse.bass as bass
import concourse.tile as tile
from concourse import bacc, mybir

N_CORES = 8
B = 4096
I = 512
O = 512
C = 10
BS = B // N_CORES          # batch rows per core (512)
NT = I // 128              # 4 i-tiles
W_SP = 2.0 / (C - 1)       # rbf width == center spacing (2/9)
IW2 = 1.0 / (W_SP * W_SP)  # 20.25
CENTERS = np.linspace(-1.0, 1.0, C)
A_J = 2.0 * CENTERS * IW2
B_J = -(CENTERS ** 2) * IW2
DB = B_J[1:] - B_J[:-1]              # 8,6,4,2,0,-2,-4,-6,-8
S_G = 64.0                           # fp8 plane scale
LN_S = float(np.log(S_G))

Q_ANCHORS = [3, 6, 8]                # via DVE q + ACT Exp
DVE_CHAIN = [(1, 0), (2, 1), (4, 3), (7, 6), (9, 8)]

_CACHE = {}
_DEBUG_TAPS = False


def _build(rank1):
    """Build and finalize the per-core Bass module (same on all cores)."""
    nc = bacc.Bacc("TRN2", target_bir_lowering=False, debug=False,
                   num_devices=N_CORES)
    f32 = mybir.dt.float32
    f16 = mybir.dt.float16
    fp8 = mybir.dt.float8e4
    DR = mybir.MatmulPerfMode.DoubleRow
    MUL = mybir.AluOpType.mult
    ADD = mybir.AluOpType.add
    EXP = mybir.ActivationFunctionType.Exp
    TANH = mybir.ActivationFunctionType.Tanh
    SQUARE = mybir.ActivationFunctionType.Square

    xt_d = nc.dram_tensor("xt", (128, NT, BS), f16, kind="ExternalInput")
    wf_d = nc.dram_tensor("wf", (128, 4 * C, O), fp8, kind="ExternalInput")
    if rank1:
        sbv_d = nc.dram_tensor("sbv", (I,), f16, kind="ExternalInput")
    else:
        ws_d = nc.dram_tensor("ws", (128, NT, O), f16, kind="ExternalInput")
    out_d = nc.dram_tensor("out", (BS, O), f32, kind="ExternalOutput")

    with tile.TileContext(nc) as tc:
        with (
            tc.tile_pool(name="big", bufs=1) as big,
            tc.tile_pool(name="wpool", bufs=1) as wpool,
            tc.tile_pool(name="psum", bufs=1, space="PSUM") as psum,
        ):
            xt_sb = big.tile([128, NT, BS], f16, tag="xt")
            xn = big.tile([128, NT, BS], f16, tag="xn")
            sq0 = big.tile([128, NT, BS], f16, tag="sq0")
            sqp = big.tile([128, NT, BS], f16, tag="sqp")
            qs = {j: big.tile([128, NT, BS], f16, name=f"q{j}")
                  for j in Q_ANCHORS}
            r_t = big.tile([128, NT, BS], f16, tag="r")
            th2 = big.tile([128, NT, BS], f16, tag="th2")
            silu = big.tile([128, NT, BS], f16, tag="silu")
            g = [big.tile([128, NT, BS], fp8, name=f"g{j}") for j in range(C)]
            wf_sb = wpool.tile([128, 4 * C, O], fp8, name="wf_sb")
            if rank1:
                sbv_sb = wpool.tile([128, NT], f16, name="sbv_sb")
                v16 = big.tile([1, BS], f16, tag="v16")
                ones_sb = big.tile([1, O], f16, tag="ones_sb")
                nc.vector.memset(ones_sb[:], 1.0)
            else:
                ws_sb = wpool.tile([128, NT, O], f16, name="ws_sb")
            warm = big.tile([128, 8], f32, tag="warm")
            gwarm = big.tile([128, 8], f16, tag="gwarm")
            # bias columns: [0]=+1 (Square shift), [1]=lnS (anchor-0 Exp),
            # [2..]= b_j + lnS for q-anchors
            bias_sb = big.tile([128, 2 + len(Q_ANCHORS)], f32, tag="bias")
            nc.vector.memset(bias_sb[:, 0:1], 1.0)
            nc.vector.memset(bias_sb[:, 1:2], LN_S)
            for ai, j in enumerate(Q_ANCHORS):
                nc.vector.memset(bias_sb[:, 2 + ai:3 + ai],
                                 float(B_J[j] + LN_S))

            def hs(h):
                return slice(2 * h, 2 * h + 2)

            # warm-ups: ACT table load + GPSIMD first-op cost at t=0
            nc.vector.memset(warm[:], 0.0)
            nc.vector.memset(gwarm[:], 0.0)
            nc.scalar.activation(out=warm[:], in_=warm[:], func=EXP)
            nc.scalar.activation(out=warm[:], in_=warm[:], func=TANH)
            nc.gpsimd.tensor_tensor(out=gwarm[:], in0=gwarm[:], in1=gwarm[:],
                                    op=MUL)
            if not rank1:
                nc.scalar.activation(out=warm[:], in_=warm[:],
                                     func=mybir.ActivationFunctionType.Silu)

            # ---- DMAs (single SP queue; bandwidth-ordered) ----
            for t in (0, 1):
                nc.sync.dma_start(out=xt_sb[:, t:t + 1, :],
                                  in_=xt_d[:, t:t + 1, :])
            if rank1:
                nc.sync.dma_start(
                    out=sbv_sb[:, :],
                    in_=sbv_d[:].rearrange("(t p) -> p t", p=128))
            # center-0 weights first so the first matmul isn't DMA-gated
            nc.sync.dma_start(out=wf_sb[:, 0:4, :], in_=wf_d[:, 0:4, :])
            for t in (2, 3):
                nc.sync.dma_start(out=xt_sb[:, t:t + 1, :],
                                  in_=xt_d[:, t:t + 1, :])
            if not rank1:
                nc.sync.dma_start(out=ws_sb[:, :, :], in_=ws_d[:, :, :])
            for (k0, k1) in [(4, 12), (12, 20), (20, 28), (28, 40)]:
                nc.sync.dma_start(out=wf_sb[:, k0:k1, :],
                                  in_=wf_d[:, k0:k1, :])

            # ---- ACT stream (one table set: tanh + exp + square) ----
            for t in range(NT):
                nc.scalar.activation(out=xn[:, t:t + 1, :],
                                     in_=xt_sb[:, t:t + 1, :], func=TANH)
            # anchor 0 fully on ACT (no DVE dependency): sq0=(xn+1)^2
            for h in (0, 1):
                nc.scalar.activation(out=sq0[:, hs(h), :],
                                     in_=xn[:, hs(h), :], func=SQUARE,
                                     bias=bias_sb[:, 0:1])
            for h in (0, 1):
                nc.scalar.activation(out=g[0][:, hs(h), :],
                                     in_=sq0[:, hs(h), :], func=EXP,
                                     scale=float(-IW2), bias=bias_sb[:, 1:2])
            for h in (0, 1):
                nc.scalar.activation(out=r_t[:, hs(h), :],
                                     in_=xn[:, hs(h), :], func=EXP,
                                     scale=float(2.0 / W_SP))
            for h in (0, 1):
                nc.scalar.activation(out=th2[:, hs(h), :],
                                     in_=xn[:, hs(h), :], func=TANH,
                                     scale=0.5)

            # ---- DVE: q's must precede the ACT exps that read them ----
            for h in (0, 1):
                nc.vector.scalar_tensor_tensor(
                    out=sqp[:, hs(h), :], in0=xn[:, hs(h), :],
                    scalar=float(-IW2), in1=xn[:, hs(h), :],
                    op0=MUL, op1=MUL)
            for j in Q_ANCHORS:
                for h in (0, 1):
                    nc.vector.scalar_tensor_tensor(
                        out=qs[j][:, hs(h), :], in0=xn[:, hs(h), :],
                        scalar=float(A_J[j]), in1=sqp[:, hs(h), :],
                        op0=MUL, op1=ADD)

            for j in Q_ANCHORS:
                ai = 2 + Q_ANCHORS.index(j)
                for h in (0, 1):
                    nc.scalar.activation(out=g[j][:, hs(h), :],
                                         in_=qs[j][:, hs(h), :], func=EXP,
                                         bias=bias_sb[:, ai:ai + 1])
            if not rank1:
                for h in (0, 1):
                    nc.scalar.activation(
                        out=silu[:, hs(h), :], in_=xn[:, hs(h), :],
                        func=mybir.ActivationFunctionType.Silu)

            def chain_op(j, src, h):
                nc.vector.scalar_tensor_tensor(
                    out=g[j][:, hs(h), :], in0=r_t[:, hs(h), :],
                    scalar=float(np.exp(DB[src])), in1=g[src][:, hs(h), :],
                    op0=MUL, op1=MUL)

            for (j, src) in [(1, 0), (2, 1), (4, 3)]:
                for h in (0, 1):
                    chain_op(j, src, h)
            # GPSIMD: (5<-4) has unit scale -> plain multiply
            assert float(np.exp(DB[4])) == 1.0
            for h in (0, 1):
                nc.gpsimd.tensor_tensor(
                    out=g[5][:, hs(h), :], in0=r_t[:, hs(h), :],
                    in1=g[4][:, hs(h), :], op=MUL)
            for (j, src) in [(7, 6), (9, 8)]:
                for h in (0, 1):
                    chain_op(j, src, h)
            if rank1:
                for h in (0, 1):
                    nc.vector.scalar_tensor_tensor(
                        out=silu[:, hs(h), :], in0=th2[:, hs(h), :],
                        scalar=1.0, in1=xn[:, hs(h), :], op0=ADD, op1=MUL)

            # ---- PE ----
            ps = [psum.tile([128, O], f32, name=f"ps{bt}") for bt in range(4)]
            if rank1:
                v_ps = psum.tile([1, BS], f32, name="v_ps")

            def mm_g(j, p, start=False, stop=False, bts=range(4)):
                for bt in bts:
                    nc.tensor.matmul(
                        ps[bt],
                        g[j][:, 2 * p:2 * p + 2, bt * 128:(bt + 1) * 128],
                        wf_sb[:, 4 * j + 2 * p:4 * j + 2 * p + 2, :],
                        start=start, stop=stop, perf_mode=DR)

            mm_g(0, 0, start=True)
            mm_g(0, 1)
            for (j, p) in [(1, 0), (1, 1), (2, 0), (2, 1), (3, 0), (3, 1),
                           (4, 0), (4, 1), (6, 0), (6, 1), (5, 0), (7, 0),
                           (5, 1), (7, 1), (8, 0), (8, 1)]:
                mm_g(j, p)
            if rank1:
                for t in range(NT):
                    nc.tensor.matmul(v_ps, sbv_sb[:, t:t + 1],
                                     silu[:, t, :],
                                     start=(t == 0), stop=(t == NT - 1))
                nc.vector.tensor_copy(out=v16[:, :], in_=v_ps[:, :])
                for bt in range(4):
                    nc.tensor.matmul(
                        ps[bt], v16[0:1, bt * 128:(bt + 1) * 128],
                        ones_sb[0:1, :], start=False, stop=False)
            else:
                for t in range(NT):
                    for bt in range(4):
                        nc.tensor.matmul(
                            ps[bt], silu[:, t, bt * 128:(bt + 1) * 128],
                            ws_sb[:, t, :], start=False, stop=False)
            mm_g(9, 0)
            # last group bank-major so bank bt drains early; out-DMAs
            # split across the SP and ACT hardware queues
            inv_s = float(1.0 / S_G)
            for bt in range(4):
                nc.tensor.matmul(
                    ps[bt], g[9][:, 2:4, bt * 128:(bt + 1) * 128],
                    wf_sb[:, 38:40, :], start=False, stop=True, perf_mode=DR)
                o_sb = big.tile([128, O], f32, name=f"o{bt}")
                if bt % 2 == 0:
                    nc.scalar.mul(out=o_sb[:], in_=ps[bt][:], mul=inv_s)
                    nc.sync.dma_start(out=out_d[bt * 128:(bt + 1) * 128, :],
                                      in_=o_sb[:])
                else:
                    nc.vector.tensor_scalar_mul(out=o_sb[:], in0=ps[bt][:],
                                                scalar1=inv_s)
                    nc.scalar.dma_start(out=out_d[bt * 128:(bt + 1) * 128, :],
                                        in_=o_sb[:])
            if _DEBUG_TAPS:
                taps = {"g0": g[0], "g1": g[1], "g5": g[5], "g9": g[9],
                        "silu": silu, "r": r_t}
                for nm, t_sb in taps.items():
                    d_out = nc.dram_tensor(f"dbg_{nm}", (128, NT, BS),
                                           t_sb.dtype, kind="ExternalOutput")
                    nc.sync.dma_start(out=d_out[:, :, :], in_=t_sb[:, :, :])
    nc.finalize()
    return nc


def _prep_inputs(x, coef, scale_base, scale_sp):
    """Host-side shard + layout prep (cheap numpy reshapes/casts)."""
    x = np.asarray(x, dtype=np.float32)
    coef = np.asarray(coef, dtype=np.float32)
    scale_base = np.asarray(scale_base, dtype=np.float32)
    scale_sp = np.asarray(scale_sp, dtype=np.float32)
    rank1 = bool(np.all(scale_base == scale_base[0:1, :]))

    # wf[p, kt, o] (partition-major for contiguous DMA), kt = 4*j + t.
    wfull = coef * scale_sp.T[:, :, None]                    # [I, O, C]
    wfull = wfull.reshape(NT, 128, O, C).transpose(3, 0, 1, 2)  # [C,NT,128,O]
    wf = np.clip(wfull.reshape(4 * C, 128, O), -240.0, 240.0).astype(
        ml_dtypes.float8_e4m3).transpose(1, 0, 2)            # [128, 4C, O]
    wf = np.ascontiguousarray(wf)

    in_maps = []
    for k in range(N_CORES):
        xs = x[k * BS:(k + 1) * BS, :]                       # [BS, I]
        xt = np.ascontiguousarray(
            xs.T.reshape(NT, 128, BS).transpose(1, 0, 2)).astype(np.float16)
        m = {"xt": xt, "wf": wf}
        if rank1:
            # silu' = 2*silu; the PSUM carries S_G everywhere, so fold
            # 0.5*S_G into sbv (v then scales down with the drain's 1/S).
            m["sbv"] = (0.5 * S_G * scale_base[0, :]).astype(np.float16)
        else:
            m["ws"] = np.ascontiguousarray(
                (S_G * scale_base.T.reshape(NT, 128, O)).transpose(1, 0, 2)
            ).astype(np.float16)
        in_maps.append(m)
    return in_maps, rank1


def _run(in_maps, rank1, trace=False):
    if "antenv.axon_hooks" not in sys.modules:
        try:
            from trn_agent_boot.trn_boot import _ntff_profile_via_ctypes
            _hook = _ntff_profile_via_ctypes("/opt/axon/libaxon_pjrt.so")
            _mod = types.ModuleType("antenv.axon_hooks")
            _mod.get_axon_ntff_profile_hook = lambda: _hook
            sys.modules["antenv.axon_hooks"] = _mod
        except Exception:
            pass
    from concourse.bass_utils import run_bass_kernel_spmd

    key = ("nc", rank1, _DEBUG_TAPS)
    if key not in _CACHE:
        _CACHE[key] = _build(rank1)
    return run_bass_kernel_spmd(_CACHE[key], in_maps,
                                core_ids=list(range(N_CORES)), trace=trace)


def kernel(x, coef, scale_base, scale_sp):
    in_maps, rank1 = _prep_inputs(x, coef, scale_base, scale_sp)
    res = _run(in_maps, rank1, trace=False)
    out = np.concatenate([res.results[k]["out"] for k in range(N_CORES)],
                         axis=0)
    return out.astype(np.float32)


# revision 17
# speedup vs baseline: 1.2436x; 1.0758x over previous
"""HeterogeneousKANLayer forward on 8 Trainium2 NeuronCores.

Math (reference):
  xn    = tanh(x)                                  [B, I]
  base  = silu(xn)                                 [B, I]
  basis = exp(-((xn - c_j)/w)^2), c_j evenly spaced on [-1,1], w = 2/(C-1)
  out[b,o] = sum_{i,c} basis[b,i,c]*coef[i,o,c]*scale_sp[o,i]
           + sum_i base[b,i]*scale_base[o,i]

Kernel strategy (data-parallel over batch, 8 cores x 512 rows):
  One [512b, 5632k] @ [5632k, 512o] contraction per core.
  The 10 Gaussian-center channel groups run as fp8e4 DoubleRow matmuls
  (K=256 per instruction, ~2x PE rate). Planes are stored scaled by S_G
  to keep the fp8 band normal; 1/S_G is folded into the PSUM drain.
  Basis production (ACT + DVE only; GPSIMD is too slow/erratic):
    - anchor 0 fully on ACT: Square(xn+1) then Exp(-20.25*sq + lnS).
    - anchors {3,6,8}: q_j = a_j*xn - 20.25*xn^2 on DVE (stt, fp16),
      g_j = Exp(q_j + b_j + lnS) on ACT directly to fp8.
    - chains (<=2 hops): g_{j+1} = (r * e^{8-2j}) * g_j on DVE, with
      r = exp(9*xn) (evenly spaced centers make consecutive Gaussians
      differ by exp(9*xn + const)).
  A dummy-matmul warm burst keeps the PE HAM clock at 2.4 GHz through
  the production-paced head. PE consumes all pair-0 halves first, then
  pair-1, so the batch-lagged second half never stalls the stream.
  silu residual via ACT Silu at the stream end (second table set).
  When scale_base is rank-1 (all rows equal, e.g. default ones), the
  residual collapses to v[b] = sum_i silu*sbv[i], computed with 4 thin
  matmuls + one K=1 matmul against a ones-row. Otherwise fall back to
  dense fp16 residual matmuls.
"""

import sys
import types

import numpy as np
import ml_dtypes

import concourse.bass as bass
import concourse.tile as tile
from concourse import bacc, mybir

N_CORES = 8
B = 4096
I = 512
O = 512
C = 10
BS = B // N_CORES          # batch rows per core (512)
NT = I // 128              # 4 i-tiles
W_SP = 2.0 / (C - 1)       # rbf width == center spacing (2/9)
IW2 = 1.0 / (W_SP * W_SP)  # 20.25
CENTERS = np.linspace(-1.0, 1.0, C)
A_J = 2.0 * CENTERS * IW2
B_J = -(CENTERS ** 2) * IW2
DB = B_J[1:] - B_J[:-1]              # 8,6,4,2,0,-2,-4,-6,-8
S_G = 64.0                           # fp8 plane scale
LN_S = float(np.log(S_G))

Q_ANCHORS = [3, 6, 8]                # via DVE q + ACT Exp
CHAINS = [(1, 0), (2, 1), (4, 3), (5, 4), (7, 6), (9, 8)]
N_WARM_MM = 24

_CACHE = {}
_DEBUG_TAPS = False


def _build(rank1):
    """Build and finalize the per-core Bass module (same on all cores)."""
    nc = bacc.Bacc("TRN2", target_bir_lowering=False, debug=False,
                   num_devices=N_CORES)
    f32 = mybir.dt.float32
    f16 = mybir.dt.float16
    fp8 = mybir.dt.float8e4
    DR = mybir.MatmulPerfMode.DoubleRow
    MUL = mybir.AluOpType.mult
    ADD = mybir.AluOpType.add
    EXP = mybir.ActivationFunctionType.Exp
    TANH = mybir.ActivationFunctionType.Tanh
    SQUARE = mybir.ActivationFunctionType.Square
    SILU = mybir.ActivationFunctionType.Silu

    xt_d = nc.dram_tensor("xt", (128, NT, BS), f16, kind="ExternalInput")
    wf_d = nc.dram_tensor("wf", (128, 4 * C, O), fp8, kind="ExternalInput")
    if rank1:
        sbv_d = nc.dram_tensor("sbv", (I,), f16, kind="ExternalInput")
    else:
        ws_d = nc.dram_tensor("ws", (128, NT, O), f16, kind="ExternalInput")
    out_d = nc.dram_tensor("out", (BS, O), f32, kind="ExternalOutput")

    with tile.TileContext(nc) as tc:
        with (
            tc.tile_pool(name="big", bufs=1) as big,
            tc.tile_pool(name="wpool", bufs=1) as wpool,
            tc.tile_pool(name="psum", bufs=1, space="PSUM") as psum,
        ):
            xt_sb = big.tile([128, NT, BS], f16, tag="xt")
            xn = big.tile([128, NT, BS], f16, tag="xn")
            sq0 = big.tile([128, NT, BS], f16, tag="sq0")
            sqp = big.tile([128, NT, BS], f16, tag="sqp")
            qs = {j: big.tile([128, NT, BS], f16, name=f"q{j}")
                  for j in Q_ANCHORS}
            r_t = big.tile([128, NT, BS], f16, tag="r")
            silu = big.tile([128, NT, BS], f16, tag="silu")
            g = [big.tile([128, NT, BS], fp8, name=f"g{j}") for j in range(C)]
            wf_sb = wpool.tile([128, 4 * C, O], fp8, name="wf_sb")
            if rank1:
                sbv_sb = wpool.tile([128, NT], f16, name="sbv_sb")
                v16 = big.tile([1, BS], f16, tag="v16")
                ones_sb = big.tile([1, O], f16, tag="ones_sb")
                nc.vector.memset(ones_sb[:], 1.0)
            else:
                ws_sb = wpool.tile([128, NT, O], f16, name="ws_sb")
            warm = big.tile([128, 8], f32, tag="warm")
            wmm_s = big.tile([128, 2, 128], fp8, tag="wmm_s")
            wmm_m = big.tile([128, 2, 512], fp8, tag="wmm_m")
            # bias columns: [0]=+1 (Square shift), [1]=lnS (anchor-0 Exp),
            # [2..]= b_j + lnS for q-anchors
            bias_sb = big.tile([128, 2 + len(Q_ANCHORS)], f32, tag="bias")
            nc.vector.memset(bias_sb[:, 0:1], 1.0)
            nc.vector.memset(bias_sb[:, 1:2], LN_S)
            for ai, j in enumerate(Q_ANCHORS):
                nc.vector.memset(bias_sb[:, 2 + ai:3 + ai],
                                 float(B_J[j] + LN_S))

            def hs(h):
                return slice(2 * h, 2 * h + 2)

            # warm-ups: ACT table load at t=0 + dummy tiles for PE burst
            nc.vector.memset(warm[:], 0.0)
            nc.scalar.activation(out=warm[:], in_=warm[:], func=EXP)
            nc.scalar.activation(out=warm[:], in_=warm[:], func=TANH)
            nc.vector.memset(wmm_s[:], 0.0)
            nc.vector.memset(wmm_m[:], 0.0)

            # ---- DMAs (single SP queue; bandwidth-ordered) ----
            for t in (0, 1):
                nc.sync.dma_start(out=xt_sb[:, t:t + 1, :],
                                  in_=xt_d[:, t:t + 1, :])
            if rank1:
                nc.sync.dma_start(
                    out=sbv_sb[:, :],
                    in_=sbv_d[:].rearrange("(t p) -> p t", p=128))
            # center-0 weights first so the first matmul isn't DMA-gated
            nc.sync.dma_start(out=wf_sb[:, 0:4, :], in_=wf_d[:, 0:4, :])
            for t in (2, 3):
                nc.sync.dma_start(out=xt_sb[:, t:t + 1, :],
                                  in_=xt_d[:, t:t + 1, :])
            if not rank1:
                nc.sync.dma_start(out=ws_sb[:, :, :], in_=ws_d[:, :, :])
            for (k0, k1) in [(4, 12), (12, 20), (20, 28), (28, 40)]:
                nc.sync.dma_start(out=wf_sb[:, k0:k1, :],
                                  in_=wf_d[:, k0:k1, :])

            # ---- PE warm burst: hold HAM at 2.4GHz through the head ----
            ps = [psum.tile([128, O], f32, name=f"ps{bt}") for bt in range(4)]
            warm_ps = psum.tile([128, O], f32, name="warm_ps")
            if rank1:
                v_ps = psum.tile([1, BS], f32, name="v_ps")
            for _ in range(N_WARM_MM):
                nc.tensor.matmul(warm_ps, wmm_s[:, :, :], wmm_m[:, :, :],
                                 start=True, stop=True, perf_mode=DR)

            # ---- production (program order == scheduler priority) ----
            for t in (0, 1):
                nc.scalar.activation(out=xn[:, t:t + 1, :],
                                     in_=xt_sb[:, t:t + 1, :], func=TANH)
            nc.scalar.activation(out=sq0[:, hs(0), :], in_=xn[:, hs(0), :],
                                 func=SQUARE, bias=bias_sb[:, 0:1])
            nc.scalar.activation(out=g[0][:, hs(0), :], in_=sq0[:, hs(0), :],
                                 func=EXP, scale=float(-IW2),
                                 bias=bias_sb[:, 1:2])
            nc.scalar.activation(out=r_t[:, hs(0), :], in_=xn[:, hs(0), :],
                                 func=EXP, scale=float(2.0 / W_SP))
            nc.vector.scalar_tensor_tensor(
                out=sqp[:, hs(0), :], in0=xn[:, hs(0), :],
                scalar=float(-IW2), in1=xn[:, hs(0), :], op0=MUL, op1=MUL)
            for t in (2, 3):
                nc.scalar.activation(out=xn[:, t:t + 1, :],
                                     in_=xt_sb[:, t:t + 1, :], func=TANH)
            nc.scalar.activation(out=sq0[:, hs(1), :], in_=xn[:, hs(1), :],
                                 func=SQUARE, bias=bias_sb[:, 0:1])
            nc.scalar.activation(out=g[0][:, hs(1), :], in_=sq0[:, hs(1), :],
                                 func=EXP, scale=float(-IW2),
                                 bias=bias_sb[:, 1:2])
            nc.scalar.activation(out=r_t[:, hs(1), :], in_=xn[:, hs(1), :],
                                 func=EXP, scale=float(2.0 / W_SP))
            nc.vector.scalar_tensor_tensor(
                out=sqp[:, hs(1), :], in0=xn[:, hs(1), :],
                scalar=float(-IW2), in1=xn[:, hs(1), :], op0=MUL, op1=MUL)
            for j in Q_ANCHORS:
                for h in (0, 1):
                    nc.vector.scalar_tensor_tensor(
                        out=qs[j][:, hs(h), :], in0=xn[:, hs(h), :],
                        scalar=float(A_J[j]), in1=sqp[:, hs(h), :],
                        op0=MUL, op1=ADD)
            for j in Q_ANCHORS:
                ai = 2 + Q_ANCHORS.index(j)
                for h in (0, 1):
                    nc.scalar.activation(out=g[j][:, hs(h), :],
                                         in_=qs[j][:, hs(h), :], func=EXP,
                                         bias=bias_sb[:, ai:ai + 1])
            for (j, src) in CHAINS:
                for h in (0, 1):
                    nc.vector.scalar_tensor_tensor(
                        out=g[j][:, hs(h), :], in0=r_t[:, hs(h), :],
                        scalar=float(np.exp(DB[src])),
                        in1=g[src][:, hs(h), :], op0=MUL, op1=MUL)
            # silu last on ACT: its table set loads after all exps are done
            for h in (0, 1):
                nc.scalar.activation(out=silu[:, hs(h), :],
                                     in_=xn[:, hs(h), :], func=SILU)

            # ---- PE: all pair-0 halves first, then pair-1 ----
            def mm_g(j, p, start=False, stop=False, bts=range(4)):
                for bt in bts:
                    nc.tensor.matmul(
                        ps[bt],
                        g[j][:, 2 * p:2 * p + 2, bt * 128:(bt + 1) * 128],
                        wf_sb[:, 4 * j + 2 * p:4 * j + 2 * p + 2, :],
                        start=start, stop=stop, perf_mode=DR)

            A_ORDER = [0, 1, 2, 3, 4, 6, 5, 7, 8, 9]
            mm_g(0, 0, start=True)
            for j in A_ORDER[1:]:
                mm_g(j, 0)
            for j in A_ORDER[:-1]:
                mm_g(j, 1)
            if rank1:
                for t in range(NT):
                    nc.tensor.matmul(v_ps, sbv_sb[:, t:t + 1],
                                     silu[:, t, :],
                                     start=(t == 0), stop=(t == NT - 1))
                nc.vector.tensor_copy(out=v16[:, :], in_=v_ps[:, :])
                for bt in range(4):
                    nc.tensor.matmul(
                        ps[bt], v16[0:1, bt * 128:(bt + 1) * 128],
                        ones_sb[0:1, :], start=False, stop=False)
            else:
                for t in range(NT):
                    for bt in range(4):
                        nc.tensor.matmul(
                            ps[bt], silu[:, t, bt * 128:(bt + 1) * 128],
                            ws_sb[:, t, :], start=False, stop=False)
            # last group bank-major so bank bt drains early; out-DMAs
            # split across the SP and ACT hardware queues
            inv_s = float(1.0 / S_G)
            for bt in range(4):
                nc.tensor.matmul(
                    ps[bt], g[9][:, 2:4, bt * 128:(bt + 1) * 128],
                    wf_sb[:, 38:40, :], start=False, stop=True, perf_mode=DR)
                o_sb = big.tile([128, O], f32, name=f"o{bt}")
                if bt % 2 == 0:
                    nc.scalar.mul(out=o_sb[:], in_=ps[bt][:], mul=inv_s)
                    nc.sync.dma_start(out=out_d[bt * 128:(bt + 1) * 128, :],
                                      in_=o_sb[:])
                else:
                    nc.vector.tensor_scalar_mul(out=o_sb[:], in0=ps[bt][:],
                                                scalar1=inv_s)
                    nc.scalar.dma_start(out=out_d[bt * 128:(bt + 1) * 128, :],
                                        in_=o_sb[:])
            if _DEBUG_TAPS:
                taps = {"g0": g[0], "g1": g[1], "g5": g[5], "g9": g[9],
                        "silu": silu, "r": r_t}
                for nm, t_sb in taps.items():
                    d_out = nc.dram_tensor(f"dbg_{nm}", (128, NT, BS),
                                           t_sb.dtype, kind="ExternalOutput")
                    nc.sync.dma_start(out=d_out[:, :, :], in_=t_sb[:, :, :])
    nc.finalize()
    return nc


def _prep_inputs(x, coef, scale_base, scale_sp):
    """Host-side shard + layout prep (cheap numpy reshapes/casts)."""
    x = np.asarray(x, dtype=np.float32)
    coef = np.asarray(coef, dtype=np.float32)
    scale_base = np.asarray(scale_base, dtype=np.float32)
    scale_sp = np.asarray(scale_sp, dtype=np.float32)
    rank1 = bool(np.all(scale_base == scale_base[0:1, :]))

    # wf[p, kt, o] (partition-major for contiguous DMA), kt = 4*j + t.
    wfull = coef * scale_sp.T[:, :, None]                    # [I, O, C]
    wfull = wfull.reshape(NT, 128, O, C).transpose(3, 0, 1, 2)  # [C,NT,128,O]
    wf = np.clip(wfull.reshape(4 * C, 128, O), -240.0, 240.0).astype(
        ml_dtypes.float8_e4m3).transpose(1, 0, 2)            # [128, 4C, O]
    wf = np.ascontiguousarray(wf)

    in_maps = []
    for k in range(N_CORES):
        xs = x[k * BS:(k + 1) * BS, :]                       # [BS, I]
        xt = np.ascontiguousarray(
            xs.T.reshape(NT, 128, BS).transpose(1, 0, 2)).astype(np.float16)
        m = {"xt": xt, "wf": wf}
        if rank1:
            # the PSUM carries S_G everywhere: fold S_G into sbv (v then
            # scales down with the drain's 1/S).
            m["sbv"] = (S_G * scale_base[0, :]).astype(np.float16)
        else:
            m["ws"] = np.ascontiguousarray(
                (S_G * scale_base.T.reshape(NT, 128, O)).transpose(1, 0, 2)
            ).astype(np.float16)
        in_maps.append(m)
    return in_maps, rank1


def _run(in_maps, rank1, trace=False):
    if "antenv.axon_hooks" not in sys.modules:
        try:
            from trn_agent_boot.trn_boot import _ntff_profile_via_ctypes
            _hook = _ntff_profile_via_ctypes("/opt/axon/libaxon_pjrt.so")
            _mod = types.ModuleType("antenv.axon_hooks")
            _mod.get_axon_ntff_profile_hook = lambda: _hook
            sys.modules["antenv.axon_hooks"] = _mod
        except Exception:
            pass
    from concourse.bass_utils import run_bass_kernel_spmd

    key = ("nc", rank1, _DEBUG_TAPS)
    if key not in _CACHE:
        _CACHE[key] = _build(rank1)
    return run_bass_kernel_spmd(_CACHE[key], in_maps,
                                core_ids=list(range(N_CORES)), trace=trace)


def kernel(x, coef, scale_base, scale_sp):
    in_maps, rank1 = _prep_inputs(x, coef, scale_base, scale_sp)
    res = _run(in_maps, rank1, trace=False)
    out = np.concatenate([res.results[k]["out"] for k in range(N_CORES)],
                         axis=0)
    return out.astype(np.float32)


# revision 18
# speedup vs baseline: 1.2457x; 1.0017x over previous
"""HeterogeneousKANLayer forward on 8 Trainium2 NeuronCores.

Math (reference):
  xn    = tanh(x)                                  [B, I]
  base  = silu(xn)                                 [B, I]
  basis = exp(-((xn - c_j)/w)^2), c_j evenly spaced on [-1,1], w = 2/(C-1)
  out[b,o] = sum_{i,c} basis[b,i,c]*coef[i,o,c]*scale_sp[o,i]
           + sum_i base[b,i]*scale_base[o,i]

Kernel strategy (data-parallel over batch, 8 cores x 512 rows):
  One [512b, 5632k] @ [5632k, 512o] contraction per core.
  The 10 Gaussian-center channel groups run as fp8e4 DoubleRow matmuls
  (K=256 per instruction, ~2x PE rate); the silu residual group runs as
  fp16 matmuls. Planes are stored scaled by S_G to keep the fp8 band
  normal; 1/S_G is folded into the PSUM drain.
  Basis production, balanced ~equally over ACT and DVE:
    - anchors 0 and 8 fully on ACT: Square(xn - c_j), Exp(-20.25*sq).
    - anchors 3 and 6 via DVE: q_j = (a_j - a_0)*xn + sq0n with
      sq0n = -20.25*sq0 - b_0 (tensor_scalar, 4x DVE mode), then
      g_j = Exp(q_j + b_j + lnS) on ACT directly to fp8.
    - chains: g_{j+1} = (r * e^{8-2j}) * g_j on DVE (stt), with
      r = exp(9*xn); evenly spaced centers make consecutive Gaussians
      differ by exp(9*xn + const). Chains 1,2 are emitted before the
      q's so the PE's early planes aren't queued behind them.
  silu = 0.5*xn*(1+tanh(xn/2)): tanh shares the exp ACT table set, so
  the whole kernel needs ONE table load; the 0.5 folds into ws.
  A dummy-matmul warm burst keeps the PE HAM clock at 2.4 GHz through
  the production-paced head. PE consumes all pair-0 halves first, then
  pair-1, so the batch-lagged second half never stalls the stream.
"""

import sys
import types

import numpy as np
import ml_dtypes

import concourse.bass as bass
import concourse.tile as tile
from concourse import bacc, mybir

N_CORES = 8
B = 4096
I = 512
O = 512
C = 10
BS = B // N_CORES          # batch rows per core (512)
NT = I // 128              # 4 i-tiles
W_SP = 2.0 / (C - 1)       # rbf width == center spacing (2/9)
IW2 = 1.0 / (W_SP * W_SP)  # 20.25
CENTERS = np.linspace(-1.0, 1.0, C)
A_J = 2.0 * CENTERS * IW2
B_J = -(CENTERS ** 2) * IW2
DB = B_J[1:] - B_J[:-1]              # 8,6,4,2,0,-2,-4,-6,-8
S_G = 64.0                           # fp8 plane scale
LN_S = float(np.log(S_G))

SQ_ANCHORS = [0, 8]                  # fully on ACT
Q_ANCHORS = [3, 6]                   # DVE q + ACT Exp
CHAINS_EARLY = [(1, 0), (2, 1)]      # feed the PE head; emitted first
CHAINS_LATE = [(4, 3), (5, 4), (7, 6), (9, 8)]
N_WARM_MM = 16

_CACHE = {}
_DEBUG_TAPS = False


def _build(rank1):
    """Build and finalize the per-core Bass module (same on all cores)."""
    nc = bacc.Bacc("TRN2", target_bir_lowering=False, debug=False,
                   num_devices=N_CORES)
    f32 = mybir.dt.float32
    f16 = mybir.dt.float16
    fp8 = mybir.dt.float8e4
    DR = mybir.MatmulPerfMode.DoubleRow
    MUL = mybir.AluOpType.mult
    ADD = mybir.AluOpType.add
    EXP = mybir.ActivationFunctionType.Exp
    TANH = mybir.ActivationFunctionType.Tanh
    SQUARE = mybir.ActivationFunctionType.Square

    xt_d = nc.dram_tensor("xt", (128, NT, BS), f16, kind="ExternalInput")
    wf_d = nc.dram_tensor("wf", (128, 4 * C, O), fp8, kind="ExternalInput")
    ws_d = nc.dram_tensor("ws", (128, NT, O), f16, kind="ExternalInput")
    out_d = nc.dram_tensor("out", (BS, O), f32, kind="ExternalOutput")

    with tile.TileContext(nc) as tc:
        with (
            tc.tile_pool(name="big", bufs=1) as big,
            tc.tile_pool(name="wpool", bufs=1) as wpool,
            tc.tile_pool(name="psum", bufs=1, space="PSUM") as psum,
        ):
            xt_sb = big.tile([128, NT, BS], f16, tag="xt")
            xn = big.tile([128, NT, BS], f16, tag="xn")
            sqa = {j: big.tile([128, NT, BS], f16, name=f"sq{j}")
                   for j in SQ_ANCHORS}
            sq0n = big.tile([128, NT, BS], f16, tag="sq0n")
            qs = {j: big.tile([128, NT, BS], f16, name=f"q{j}")
                  for j in Q_ANCHORS}
            r_t = big.tile([128, NT, BS], f16, tag="r")
            th2 = big.tile([128, NT, BS], f16, tag="th2")
            silu = big.tile([128, NT, BS], f16, tag="silu")
            g = [big.tile([128, NT, BS], fp8, name=f"g{j}") for j in range(C)]
            wf_sb = wpool.tile([128, 4 * C, O], fp8, name="wf_sb")
            ws_sb = wpool.tile([128, NT, O], f16, name="ws_sb")
            warm = big.tile([128, 8], f32, tag="warm")
            wmm_s = big.tile([128, 2, 128], fp8, tag="wmm_s")
            wmm_m = big.tile([128, 2, 512], fp8, tag="wmm_m")
            # bias columns: [0]=-c_0, [1]=-c_8 (Square shifts),
            # [2]=lnS (sq-anchor Exp), [3..]= b_j + lnS for q-anchors
            bias_sb = big.tile([128, 3 + len(Q_ANCHORS)], f32, tag="bias")
            nc.vector.memset(bias_sb[:, 0:1], float(-CENTERS[0]))
            nc.vector.memset(bias_sb[:, 1:2], float(-CENTERS[8]))
            nc.vector.memset(bias_sb[:, 2:3], LN_S)
            for ai, j in enumerate(Q_ANCHORS):
                nc.vector.memset(bias_sb[:, 3 + ai:4 + ai],
                                 float(B_J[j] + LN_S))

            def hs(h):
                return slice(2 * h, 2 * h + 2)

            # warm-ups: ACT table load at t=0 + dummy tiles for PE burst
            nc.vector.memset(warm[:], 0.0)
            nc.scalar.activation(out=warm[:], in_=warm[:], func=EXP)
            nc.scalar.activation(out=warm[:], in_=warm[:], func=TANH)
            nc.vector.memset(wmm_s[:], 0.0)
            nc.vector.memset(wmm_m[:], 0.0)

            # ---- DMAs (single SP queue; bandwidth-ordered) ----
            for t in (0, 1):
                nc.sync.dma_start(out=xt_sb[:, t:t + 1, :],
                                  in_=xt_d[:, t:t + 1, :])
            # center-0 weights first so the first matmul isn't DMA-gated
            nc.sync.dma_start(out=wf_sb[:, 0:4, :], in_=wf_d[:, 0:4, :])
            for t in (2, 3):
                nc.sync.dma_start(out=xt_sb[:, t:t + 1, :],
                                  in_=xt_d[:, t:t + 1, :])
            nc.sync.dma_start(out=wf_sb[:, 4:12, :], in_=wf_d[:, 4:12, :])
            nc.sync.dma_start(out=ws_sb[:, :, :], in_=ws_d[:, :, :])
            for (k0, k1) in [(12, 20), (20, 28), (28, 40)]:
                nc.sync.dma_start(out=wf_sb[:, k0:k1, :],
                                  in_=wf_d[:, k0:k1, :])

            # ---- PE warm burst: hold HAM at 2.4GHz through the head ----
            ps = [psum.tile([128, O], f32, name=f"ps{bt}") for bt in range(4)]
            warm_ps = psum.tile([128, O], f32, name="warm_ps")
            for _ in range(N_WARM_MM):
                nc.tensor.matmul(warm_ps, wmm_s[:, :, :], wmm_m[:, :, :],
                                 start=True, stop=True, perf_mode=DR)

            # ---- production (program order == scheduler priority) ----
            def act(out, in_, func, h, **kw):
                nc.scalar.activation(out=out[:, hs(h), :],
                                     in_=in_[:, hs(h), :], func=func, **kw)

            def stt(out, in0, scalar, in1, h, op0=MUL, op1=MUL):
                nc.vector.scalar_tensor_tensor(
                    out=out[:, hs(h), :], in0=in0[:, hs(h), :],
                    scalar=scalar, in1=in1[:, hs(h), :], op0=op0, op1=op1)

            def chain(j, src, h):
                stt(g[j], r_t, float(np.exp(DB[src])), g[src], h)

            # half A head: tanh -> g0 -> r -> early chains
            for t in (0, 1):
                nc.scalar.activation(out=xn[:, t:t + 1, :],
                                     in_=xt_sb[:, t:t + 1, :], func=TANH)
            act(sqa[0], xn, SQUARE, 0, bias=bias_sb[:, 0:1])
            act(g[0], sqa[0], EXP, 0, scale=float(-IW2), bias=bias_sb[:, 2:3])
            act(r_t, xn, EXP, 0, scale=float(2.0 / W_SP))
            nc.vector.tensor_scalar(
                out=sq0n[:, hs(0), :], in0=sqa[0][:, hs(0), :],
                scalar1=float(-IW2), scalar2=float(-B_J[0]),
                op0=MUL, op1=ADD)
            chain(1, 0, 0)
            chain(2, 1, 0)
            # half B head
            for t in (2, 3):
                nc.scalar.activation(out=xn[:, t:t + 1, :],
                                     in_=xt_sb[:, t:t + 1, :], func=TANH)
            act(sqa[0], xn, SQUARE, 1, bias=bias_sb[:, 0:1])
            act(g[0], sqa[0], EXP, 1, scale=float(-IW2), bias=bias_sb[:, 2:3])
            act(r_t, xn, EXP, 1, scale=float(2.0 / W_SP))
            nc.vector.tensor_scalar(
                out=sq0n[:, hs(1), :], in0=sqa[0][:, hs(1), :],
                scalar1=float(-IW2), scalar2=float(-B_J[0]),
                op0=MUL, op1=ADD)
            chain(1, 0, 1)
            chain(2, 1, 1)
            # q-anchors 3,6 (DVE) -> ACT exps; sq-anchor 8 on ACT
            for j in Q_ANCHORS:
                for h in (0, 1):
                    stt(qs[j], xn, float(A_J[j] - A_J[0]), sq0n, h,
                        op0=MUL, op1=ADD)
            for h in (0, 1):
                act(qs_exp_out := g[Q_ANCHORS[0]], qs[Q_ANCHORS[0]], EXP, h,
                    bias=bias_sb[:, 3:4])
            for h in (0, 1):
                act(sqa[8], xn, SQUARE, h, bias=bias_sb[:, 1:2])
                act(g[8], sqa[8], EXP, h, scale=float(-IW2),
                    bias=bias_sb[:, 2:3])
            for h in (0, 1):
                act(g[Q_ANCHORS[1]], qs[Q_ANCHORS[1]], EXP, h,
                    bias=bias_sb[:, 4:5])
            # th2 -> silu' = xn*(1+th2) = 2*silu (0.5 folded into ws)
            for h in (0, 1):
                act(th2, xn, TANH, h, scale=0.5)
            for (j, src) in [(4, 3)]:
                for h in (0, 1):
                    chain(j, src, h)
            for h in (0, 1):
                stt(silu, th2, 1.0, xn, h, op0=ADD, op1=MUL)
            for (j, src) in [(5, 4), (7, 6), (9, 8)]:
                for h in (0, 1):
                    chain(j, src, h)

            # ---- PE: all pair-0 halves first, then pair-1 ----
            def mm_g(j, p, start=False, stop=False):
                for bt in range(4):
                    nc.tensor.matmul(
                        ps[bt],
                        g[j][:, 2 * p:2 * p + 2, bt * 128:(bt + 1) * 128],
                        wf_sb[:, 4 * j + 2 * p:4 * j + 2 * p + 2, :],
                        start=start, stop=stop, perf_mode=DR)

            def mm_silu(t):
                for bt in range(4):
                    nc.tensor.matmul(
                        ps[bt], silu[:, t, bt * 128:(bt + 1) * 128],
                        ws_sb[:, t, :], start=False, stop=False)

            A_ORDER = [0, 1, 2, 3, 4, 6, 8, 5, 7, 9]
            mm_g(0, 0, start=True)
            for j in A_ORDER[1:]:
                mm_g(j, 0)
            mm_silu(0)
            mm_silu(1)
            for j in A_ORDER[:-1]:
                mm_g(j, 1)
            mm_silu(2)
            mm_silu(3)
            # last group bank-major so bank bt drains early; out-DMAs
            # split across the SP and ACT hardware queues
            inv_s = float(1.0 / S_G)
            for bt in range(4):
                nc.tensor.matmul(
                    ps[bt], g[9][:, 2:4, bt * 128:(bt + 1) * 128],
                    wf_sb[:, 38:40, :], start=False, stop=True, perf_mode=DR)
                o_sb = big.tile([128, O], f32, name=f"o{bt}")
                if bt % 2 == 0:
                    nc.scalar.mul(out=o_sb[:], in_=ps[bt][:], mul=inv_s)
                    nc.sync.dma_start(out=out_d[bt * 128:(bt + 1) * 128, :],
                                      in_=o_sb[:])
                else:
                    nc.vector.tensor_scalar_mul(out=o_sb[:], in0=ps[bt][:],
                                                scalar1=inv_s)
                    nc.scalar.dma_start(out=out_d[bt * 128:(bt + 1) * 128, :],
                                        in_=o_sb[:])
            if _DEBUG_TAPS:
                taps = {"g0": g[0], "g1": g[1], "g5": g[5], "g9": g[9],
                        "silu": silu, "r": r_t}
                for nm, t_sb in taps.items():
                    d_out = nc.dram_tensor(f"dbg_{nm}", (128, NT, BS),
                                           t_sb.dtype, kind="ExternalOutput")
                    nc.sync.dma_start(out=d_out[:, :, :], in_=t_sb[:, :, :])
    nc.finalize()
    return nc


def _prep_inputs(x, coef, scale_base, scale_sp):
    """Host-side shard + layout prep (cheap numpy reshapes/casts)."""
    x = np.asarray(x, dtype=np.float32)
    coef = np.asarray(coef, dtype=np.float32)
    scale_base = np.asarray(scale_base, dtype=np.float32)
    scale_sp = np.asarray(scale_sp, dtype=np.float32)

    # wf[p, kt, o] (partition-major for contiguous DMA), kt = 4*j + t.
    wfull = coef * scale_sp.T[:, :, None]                    # [I, O, C]
    wfull = wfull.reshape(NT, 128, O, C).transpose(3, 0, 1, 2)  # [C,NT,128,O]
    wf = np.clip(wfull.reshape(4 * C, 128, O), -240.0, 240.0).astype(
        ml_dtypes.float8_e4m3).transpose(1, 0, 2)            # [128, 4C, O]
    wf = np.ascontiguousarray(wf)
    # silu' = 2*silu and the PSUM carries S_G: fold 0.5*S_G into ws.
    ws = np.ascontiguousarray(
        (0.5 * S_G * scale_base.T.reshape(NT, 128, O)).transpose(1, 0, 2)
    ).astype(np.float16)

    in_maps = []
    for k in range(N_CORES):
        xs = x[k * BS:(k + 1) * BS, :]                       # [BS, I]
        xt = np.ascontiguousarray(
            xs.T.reshape(NT, 128, BS).transpose(1, 0, 2)).astype(np.float16)
        in_maps.append({"xt": xt, "wf": wf, "ws": ws})
    return in_maps, True


def _run(in_maps, rank1, trace=False):
    if "antenv.axon_hooks" not in sys.modules:
        try:
            from trn_agent_boot.trn_boot import _ntff_profile_via_ctypes
            _hook = _ntff_profile_via_ctypes("/opt/axon/libaxon_pjrt.so")
            _mod = types.ModuleType("antenv.axon_hooks")
            _mod.get_axon_ntff_profile_hook = lambda: _hook
            sys.modules["antenv.axon_hooks"] = _mod
        except Exception:
            pass
    from concourse.bass_utils import run_bass_kernel_spmd

    key = ("nc", _DEBUG_TAPS)
    if key not in _CACHE:
        _CACHE[key] = _build(True)
    return run_bass_kernel_spmd(_CACHE[key], in_maps,
                                core_ids=list(range(N_CORES)), trace=trace)


def kernel(x, coef, scale_base, scale_sp):
    in_maps, rank1 = _prep_inputs(x, coef, scale_base, scale_sp)
    res = _run(in_maps, rank1, trace=False)
    out = np.concatenate([res.results[k]["out"] for k in range(N_CORES)],
                         axis=0)
    return out.astype(np.float32)


# revision 21
# speedup vs baseline: 1.2651x; 1.0156x over previous
"""HeterogeneousKANLayer forward on 8 Trainium2 NeuronCores.

Math (reference):
  xn    = tanh(x)                                  [B, I]
  base  = silu(xn)                                 [B, I]
  basis = exp(-((xn - c_j)/w)^2), c_j evenly spaced on [-1,1], w = 2/(C-1)
  out[b,o] = sum_{i,c} basis[b,i,c]*coef[i,o,c]*scale_sp[o,i]
           + sum_i base[b,i]*scale_base[o,i]

Kernel strategy (data-parallel over batch, 8 cores x 512 rows):
  One [512b, 5632k] @ [5632k, 512o] contraction per core.
  The 10 Gaussian-center channel groups run as fp8e4 DoubleRow matmuls
  (K=256 per instruction, ~2x PE rate); the silu residual group runs as
  fp16 matmuls. Planes are stored scaled by S_G to keep the fp8 band
  normal; 1/S_G is folded into the PSUM drain.
  Basis production, balanced ~equally over ACT and DVE:
    - anchors 0 and 8 fully on ACT: Square(xn - c_j), Exp(-20.25*sq).
    - anchors 3 and 6 via DVE: q_j = (a_j - a_0)*xn + sq0n with
      sq0n = -20.25*sq0 - b_0 (tensor_scalar, 4x DVE mode), then
      g_j = Exp(q_j + b_j + lnS) on ACT directly to fp8.
    - chains: g_{j+1} = (r * e^{8-2j}) * g_j on DVE (stt), with
      r = exp(9*xn); evenly spaced centers make consecutive Gaussians
      differ by exp(9*xn + const). Chains 1,2 are emitted before the
      q's so the PE's early planes aren't queued behind them.
  silu = 0.5*xn*(1+tanh(xn/2)): tanh shares the exp ACT table set, so
  the whole kernel needs ONE table load; the 0.5 folds into ws.
  A dummy-matmul warm burst keeps the PE HAM clock at 2.4 GHz through
  the production-paced head. PE consumes all pair-0 halves first, then
  pair-1, so the batch-lagged second half never stalls the stream.
"""

import sys
import types

import numpy as np
import ml_dtypes

import concourse.bass as bass
import concourse.tile as tile
from concourse import bacc, mybir

N_CORES = 8
B = 4096
I = 512
O = 512
C = 10
BS = B // N_CORES          # batch rows per core (512)
NT = I // 128              # 4 i-tiles
W_SP = 2.0 / (C - 1)       # rbf width == center spacing (2/9)
IW2 = 1.0 / (W_SP * W_SP)  # 20.25
CENTERS = np.linspace(-1.0, 1.0, C)
A_J = 2.0 * CENTERS * IW2
B_J = -(CENTERS ** 2) * IW2
DB = B_J[1:] - B_J[:-1]              # 8,6,4,2,0,-2,-4,-6,-8
S_G = 64.0                           # fp8 plane scale
LN_S = float(np.log(S_G))

SQ_ANCHORS = [0, 3]                  # fully on ACT
Q_ANCHORS = [6, 8]                   # DVE q + ACT Exp
CHAINS_EARLY = [(1, 0), (2, 1)]      # feed the PE head; emitted first
CHAINS_LATE = [(4, 3), (5, 4), (7, 6), (9, 8)]
N_WARM_MM = 14

_CACHE = {}
_DEBUG_TAPS = False


def _build(rank1):
    """Build and finalize the per-core Bass module (same on all cores)."""
    nc = bacc.Bacc("TRN2", target_bir_lowering=False, debug=False,
                   num_devices=N_CORES)
    f32 = mybir.dt.float32
    f16 = mybir.dt.float16
    fp8 = mybir.dt.float8e4
    DR = mybir.MatmulPerfMode.DoubleRow
    MUL = mybir.AluOpType.mult
    ADD = mybir.AluOpType.add
    EXP = mybir.ActivationFunctionType.Exp
    TANH = mybir.ActivationFunctionType.Tanh
    SQUARE = mybir.ActivationFunctionType.Square

    xt_d = nc.dram_tensor("xt", (128, NT, BS), f16, kind="ExternalInput")
    wf_d = nc.dram_tensor("wf", (128, 4 * C, O), fp8, kind="ExternalInput")
    ws_d = nc.dram_tensor("ws", (128, NT, O), f16, kind="ExternalInput")
    out_d = nc.dram_tensor("out", (BS, O), f32, kind="ExternalOutput")

    with tile.TileContext(nc) as tc:
        with (
            tc.tile_pool(name="big", bufs=1) as big,
            tc.tile_pool(name="wpool", bufs=1) as wpool,
            tc.tile_pool(name="psum", bufs=1, space="PSUM") as psum,
        ):
            xt_sb = big.tile([128, NT, BS], f16, tag="xt")
            xn = big.tile([128, NT, BS], f16, tag="xn")
            sqa = {j: big.tile([128, NT, BS], f16, name=f"sq{j}")
                   for j in SQ_ANCHORS}
            sq0n = big.tile([128, NT, BS], f16, tag="sq0n")
            qs = {j: big.tile([128, NT, BS], f16, name=f"q{j}")
                  for j in Q_ANCHORS}
            r_t = big.tile([128, NT, BS], f16, tag="r")
            th2 = big.tile([128, NT, BS], f16, tag="th2")
            silu = big.tile([128, NT, BS], f16, tag="silu")
            g = [big.tile([128, NT, BS], fp8, name=f"g{j}") for j in range(C)]
            wf_sb = wpool.tile([128, 4 * C, O], fp8, name="wf_sb")
            ws_sb = wpool.tile([128, NT, O], f16, name="ws_sb")
            warm = big.tile([128, 8], f32, tag="warm")
            dwarm = big.tile([128, 1], f16, tag="dwarm")
            wmm_s = big.tile([128, 2, 128], fp8, tag="wmm_s")
            wmm_m = big.tile([128, 2, 512], fp8, tag="wmm_m")
            # bias columns: [0]=-c_0, [1]=-c_8 (Square shifts),
            # [2]=lnS (sq-anchor Exp), [3..]= b_j + lnS for q-anchors
            bias_sb = big.tile([128, 3 + len(Q_ANCHORS)], f32, tag="bias")
            nc.vector.memset(bias_sb[:, 0:1], float(-CENTERS[0]))
            nc.vector.memset(bias_sb[:, 1:2], float(-CENTERS[3]))
            nc.vector.memset(bias_sb[:, 2:3], LN_S)
            for ai, j in enumerate(Q_ANCHORS):
                nc.vector.memset(bias_sb[:, 3 + ai:4 + ai],
                                 float(B_J[j] + LN_S))

            def hs(h):
                return slice(2 * h, 2 * h + 2)

            # warm-ups: ACT table load at t=0 + dummy tiles for PE burst
            nc.vector.memset(warm[:], 0.0)
            nc.scalar.activation(out=warm[:], in_=warm[:], func=EXP)
            nc.scalar.activation(out=warm[:], in_=warm[:], func=TANH)
            nc.vector.memset(wmm_s[:], 0.0)
            nc.vector.memset(wmm_m[:], 0.0)

            # ---- DMAs (single SP queue; bandwidth-ordered) ----
            nc.sync.dma_start(out=dwarm[:, 0:1], in_=xt_d[0:128, 0:1, 0:1])
            for t in (0, 1):
                nc.sync.dma_start(out=xt_sb[:, t:t + 1, :],
                                  in_=xt_d[:, t:t + 1, :])
            # center-0 weights first so the first matmul isn't DMA-gated
            nc.sync.dma_start(out=wf_sb[:, 0:4, :], in_=wf_d[:, 0:4, :])
            for t in (2, 3):
                nc.sync.dma_start(out=xt_sb[:, t:t + 1, :],
                                  in_=xt_d[:, t:t + 1, :])
            nc.sync.dma_start(out=wf_sb[:, 4:12, :], in_=wf_d[:, 4:12, :])
            nc.sync.dma_start(out=ws_sb[:, :, :], in_=ws_d[:, :, :])
            for (k0, k1) in [(12, 20), (20, 28), (28, 40)]:
                nc.sync.dma_start(out=wf_sb[:, k0:k1, :],
                                  in_=wf_d[:, k0:k1, :])

            # ---- PE warm burst: hold HAM at 2.4GHz through the head ----
            ps = [psum.tile([128, O], f32, name=f"ps{bt}") for bt in range(4)]
            warm_ps = psum.tile([128, O], f32, name="warm_ps")
            for _ in range(N_WARM_MM):
                nc.tensor.matmul(warm_ps, wmm_s[:, :, :], wmm_m[:, :, :],
                                 start=True, stop=True, perf_mode=DR)

            # ---- production (program order == scheduler priority) ----
            def act(out, in_, func, h, **kw):
                nc.scalar.activation(out=out[:, hs(h), :],
                                     in_=in_[:, hs(h), :], func=func, **kw)

            def stt(out, in0, scalar, in1, h, op0=MUL, op1=MUL):
                nc.vector.scalar_tensor_tensor(
                    out=out[:, hs(h), :], in0=in0[:, hs(h), :],
                    scalar=scalar, in1=in1[:, hs(h), :], op0=op0, op1=op1)

            def chain(j, src, h):
                stt(g[j], r_t, float(np.exp(DB[src])), g[src], h)

            def sq_anchor(j, bias_col, h):
                act(sqa[j], xn, SQUARE, h, bias=bias_sb[:, bias_col:bias_col + 1])
                act(g[j], sqa[j], EXP, h, scale=float(-IW2),
                    bias=bias_sb[:, 2:3])

            # --- half A: ACT head then DVE block ---
            for t in (0, 1):
                nc.scalar.activation(out=xn[:, t:t + 1, :],
                                     in_=xt_sb[:, t:t + 1, :], func=TANH)
            sq_anchor(0, 0, 0)
            act(r_t, xn, EXP, 0, scale=float(2.0 / W_SP))
            sq_anchor(3, 1, 0)
            chain(1, 0, 0)
            chain(2, 1, 0)
            nc.vector.tensor_scalar(
                out=sq0n[:, hs(0), :], in0=sqa[0][:, hs(0), :],
                scalar1=float(-IW2), scalar2=float(-B_J[0]),
                op0=MUL, op1=ADD)
            for j in Q_ANCHORS:
                stt(qs[j], xn, float(A_J[j] - A_J[0]), sq0n, 0,
                    op0=MUL, op1=ADD)
            chain(4, 3, 0)
            chain(5, 4, 0)
            # --- half B: ACT head then DVE block ---
            for t in (2, 3):
                nc.scalar.activation(out=xn[:, t:t + 1, :],
                                     in_=xt_sb[:, t:t + 1, :], func=TANH)
            sq_anchor(0, 0, 1)
            act(r_t, xn, EXP, 1, scale=float(2.0 / W_SP))
            sq_anchor(3, 1, 1)
            chain(1, 0, 1)
            chain(2, 1, 1)
            nc.vector.tensor_scalar(
                out=sq0n[:, hs(1), :], in0=sqa[0][:, hs(1), :],
                scalar1=float(-IW2), scalar2=float(-B_J[0]),
                op0=MUL, op1=ADD)
            for j in Q_ANCHORS:
                stt(qs[j], xn, float(A_J[j] - A_J[0]), sq0n, 1,
                    op0=MUL, op1=ADD)
            chain(4, 3, 1)
            chain(5, 4, 1)
            # --- q-anchor exps, th2, late chains, silu ---
            for j in Q_ANCHORS:
                ai = 3 + Q_ANCHORS.index(j)
                for h in (0, 1):
                    act(g[j], qs[j], EXP, h, bias=bias_sb[:, ai:ai + 1])
            for h in (0, 1):
                act(th2, xn, TANH, h, scale=0.5)
            for h in (0, 1):
                chain(7, 6, h)
            for h in (0, 1):
                chain(9, 8, h)
            for h in (0, 1):
                stt(silu, th2, 1.0, xn, h, op0=ADD, op1=MUL)

            # ---- PE: pair-0 pass, pair-1 pass, then silu as the stop ----
            def mm_g(j, p, start=False, stop=False):
                for bt in range(4):
                    nc.tensor.matmul(
                        ps[bt],
                        g[j][:, 2 * p:2 * p + 2, bt * 128:(bt + 1) * 128],
                        wf_sb[:, 4 * j + 2 * p:4 * j + 2 * p + 2, :],
                        start=start, stop=stop, perf_mode=DR)

            ORDER = [0, 1, 2, 3, 4, 5, 6, 8, 7, 9]
            mm_g(0, 0, start=True)
            for j in ORDER[1:]:
                mm_g(j, 0)
            for j in ORDER:
                mm_g(j, 1)
            for t in (0, 1, 2):
                for bt in range(4):
                    nc.tensor.matmul(
                        ps[bt], silu[:, t, bt * 128:(bt + 1) * 128],
                        ws_sb[:, t, :], start=False, stop=False)
            # silu t3 bank-major with stop so bank bt drains early;
            # out-DMAs split across the SP and ACT hardware queues
            inv_s = float(1.0 / S_G)
            for bt in range(4):
                nc.tensor.matmul(
                    ps[bt], silu[:, 3, bt * 128:(bt + 1) * 128],
                    ws_sb[:, 3, :], start=False, stop=True)
                o_sb = big.tile([128, O], f32, name=f"o{bt}")
                if bt % 2 == 0:
                    nc.scalar.mul(out=o_sb[:], in_=ps[bt][:], mul=inv_s)
                    nc.sync.dma_start(out=out_d[bt * 128:(bt + 1) * 128, :],
                                      in_=o_sb[:])
                else:
                    nc.vector.tensor_scalar_mul(out=o_sb[:], in0=ps[bt][:],
                                                scalar1=inv_s)
                    nc.scalar.dma_start(out=out_d[bt * 128:(bt + 1) * 128, :],
                                        in_=o_sb[:])
            if _DEBUG_TAPS:
                taps = {"g0": g[0], "g1": g[1], "g5": g[5], "g9": g[9],
                        "silu": silu, "r": r_t}
                for nm, t_sb in taps.items():
                    d_out = nc.dram_tensor(f"dbg_{nm}", (128, NT, BS),
                                           t_sb.dtype, kind="ExternalOutput")
                    nc.sync.dma_start(out=d_out[:, :, :], in_=t_sb[:, :, :])
    nc.finalize()
    return nc


def _prep_inputs(x, coef, scale_base, scale_sp):
    """Host-side shard + layout prep (cheap numpy reshapes/casts)."""
    x = np.asarray(x, dtype=np.float32)
    coef = np.asarray(coef, dtype=np.float32)
    scale_base = np.asarray(scale_base, dtype=np.float32)
    scale_sp = np.asarray(scale_sp, dtype=np.float32)

    # wf[p, kt, o] (partition-major for contiguous DMA), kt = 4*j + t.
    wfull = coef * scale_sp.T[:, :, None]                    # [I, O, C]
    wfull = wfull.reshape(NT, 128, O, C).transpose(3, 0, 1, 2)  # [C,NT,128,O]
    wf = np.clip(wfull.reshape(4 * C, 128, O), -240.0, 240.0).astype(
        ml_dtypes.float8_e4m3).transpose(1, 0, 2)            # [128, 4C, O]
    wf = np.ascontiguousarray(wf)
    # silu' = 2*silu and the PSUM carries S_G: fold 0.5*S_G into ws.
    ws = np.ascontiguousarray(
        (0.5 * S_G * scale_base.T.reshape(NT, 128, O)).transpose(1, 0, 2)
    ).astype(np.float16)

    in_maps = []
    for k in range(N_CORES):
        xs = x[k * BS:(k + 1) * BS, :]                       # [BS, I]
        xt = np.ascontiguousarray(
            xs.T.reshape(NT, 128, BS).transpose(1, 0, 2)).astype(np.float16)
        in_maps.append({"xt": xt, "wf": wf, "ws": ws})
    return in_maps, True


def _run(in_maps, rank1, trace=False):
    if "antenv.axon_hooks" not in sys.modules:
        try:
            from trn_agent_boot.trn_boot import _ntff_profile_via_ctypes
            _hook = _ntff_profile_via_ctypes("/opt/axon/libaxon_pjrt.so")
            _mod = types.ModuleType("antenv.axon_hooks")
            _mod.get_axon_ntff_profile_hook = lambda: _hook
            sys.modules["antenv.axon_hooks"] = _mod
        except Exception:
            pass
    from concourse.bass_utils import run_bass_kernel_spmd

    key = ("nc", _DEBUG_TAPS)
    if key not in _CACHE:
        _CACHE[key] = _build(True)
    return run_bass_kernel_spmd(_CACHE[key], in_maps,
                                core_ids=list(range(N_CORES)), trace=trace)


def kernel(x, coef, scale_base, scale_sp):
    in_maps, rank1 = _prep_inputs(x, coef, scale_base, scale_sp)
    res = _run(in_maps, rank1, trace=False)
    out = np.concatenate([res.results[k]["out"] for k in range(N_CORES)],
                         axis=0)
    return out.astype(np.float32)
